# revision 3
# baseline (speedup 1.0000x reference)
"""Multi-head attention forward on 8 TRN2 NeuronCores.

Sharding: tensor-parallel over heads (4 groups of 4 heads) x data-parallel
over batch (2). Core c: batch c//4, heads [4*(c%4), 4*(c%4)+4).
Each 4-core batch group ReduceScatters the projection partials so every
core ends with a disjoint [4, 128, 1024] slice of the final output; the
host reassembles the full [2, 2048, 1024].

Compute layout is feature-major (transposed) throughout:
  qkvT = W_shard^T @ x^T          [768, T]   (PE, bf16 in / f32 psum)
  S^T  = kT^T q T per k-tile      [128, TQC] (PE)
  P^T  = exp(S^T / 64)            (ScalarE; no max-subtraction needed:
                                   scores have sigma ~0.125)
  O_aug^T = V_aug^T @ P^T accum   [65, TQC]  (V_aug has a ones column so
                                   row 64 accumulates the softmax denom)
  transpose 128-col blocks of O_aug -> q on partitions -> reciprocal *
  scale on VectorE -> transpose back -> O_all^T
  y = O_all^T^T @ W_proj          [128, 1024] psum per row-tile
  chunked ReduceScatter(4 ranks) of y overlaps next chunk's attention.
"""
import os
import sys
import types

import numpy as np

if "/opt/trn_rl_repo" not in sys.path:
    sys.path.insert(0, "/opt/trn_rl_repo")

import concourse.bass as bass
import concourse.bacc as bacc
import concourse.tile as tile
import concourse.mybir as mybir
from concourse import masks
from concourse.bass_utils import run_bass_kernel_spmd

B, T, D = 2, 2048, 1024
H, HD = 16, 64
N_CORES = 8
GROUPS = [[0, 1, 2, 3], [4, 5, 6, 7]]
HPC = 4                 # heads per core
DSH = HPC * HD          # 256 per-core head features
QKV_COLS = 3 * DSH      # 768
TQC = 512               # q-chunk
N_TQ = T // TQC         # 4
N_KT = T // 128         # 16 k-tiles

f32 = mybir.dt.float32
bf16 = mybir.dt.bfloat16

LAST_EXEC_NS = None
_CACHE = {}


def _build():
    nc = bacc.Bacc("TRN2", target_bir_lowering=False, debug=False,
                   num_devices=N_CORES)
    xT_ext = nc.dram_tensor("xT", [D, T], bf16, kind="ExternalInput")
    wqkv_ext = nc.dram_tensor("w_qkv", [D, QKV_COLS], bf16,
                              kind="ExternalInput")
    wproj_ext = nc.dram_tensor("w_proj", [DSH, D], bf16, kind="ExternalInput")
    out_ext = nc.dram_tensor("out", [N_TQ, TQC // 4, D], f32,
                             kind="ExternalOutput")
    Exp = mybir.ActivationFunctionType.Exp

    with tile.TileContext(nc) as tc:
        with (
            tc.tile_pool(name="persist", bufs=1) as persist,
            tc.tile_pool(name="dram", bufs=1, space="DRAM") as drampool,
        ):
            ident = persist.tile([128, 128], bf16)
            masks.make_identity(nc, ident[:])

            qkvT = persist.tile([128, 6, T], bf16)   # rows m*128+p of [768,T]
            wproj = persist.tile([128, 2, D], bf16)  # rows of [256, 1024]
            oallT = persist.tile([128, 2, T], bf16)  # rows of O_all^T [256, T]
            vaug = persist.tile([128, HPC, N_KT, HD + 1], bf16)

            nc.sync.dma_start(
                wproj[:], wproj_ext.ap().rearrange("(c p) d -> p c d", p=128))

            y_bounce = drampool.tile([T, D], f32, tag="ybounce")

            # ---- QKV: qkvT = W_qkv_shard^T @ x^T ----
            with (
                tc.tile_pool(name="qkv_in", bufs=1) as qin,
                tc.tile_pool(name="ps_qkv", bufs=2, space="PSUM") as psq,
            ):
                xT = qin.tile([128, 8, T], bf16)
                wqkv = qin.tile([128, 8, QKV_COLS], bf16)
                nc.sync.dma_start(
                    xT[:], xT_ext.ap().rearrange("(k p) t -> p k t", p=128))
                nc.sync.dma_start(
                    wqkv[:], wqkv_ext.ap().rearrange("(k p) m -> p k m", p=128))
                for tch in range(N_TQ):
                    t0 = tch * TQC
                    for m in range(6):
                        ps = psq.tile([128, TQC], f32, tag="qkv")
                        for k in range(8):
                            nc.tensor.matmul(
                                ps[:],
                                wqkv[:, k, m * 128:(m + 1) * 128],
                                xT[:, k, t0:t0 + TQC],
                                start=(k == 0), stop=(k == 7),
                            )
                        nc.vector.tensor_copy(qkvT[:, m, t0:t0 + TQC], ps[:])

            # ---- V_aug: per head, transposed V tiles with ones column ----
            with tc.tile_pool(name="ps_vtr", bufs=2, space="PSUM") as psv:
                nc.gpsimd.memset(vaug[:, :, :, HD:HD + 1], 1.0)
                for h in range(HPC):
                    vm, vp = 4 + h // 2, (h % 2) * 64
                    for kt in range(N_KT):
                        ptr = psv.tile([128, HD], bf16, tag="vtr")
                        nc.tensor.transpose(
                            ptr[:],
                            qkvT[vp:vp + HD, vm, kt * 128:(kt + 1) * 128],
                            ident[vp:vp + HD, vp:vp + HD])
                        nc.vector.tensor_copy(vaug[:, h, kt, 0:HD], ptr[:])

            # ---- attention + proj + chunked reduce-scatter ----
            with (
                tc.tile_pool(name="ps_s", bufs=2, space="PSUM") as pss,
                tc.tile_pool(name="ps_o", bufs=2, space="PSUM") as pso,
                tc.tile_pool(name="ps_t", bufs=2, space="PSUM") as pst,
                tc.tile_pool(name="ps_y", bufs=2, space="PSUM") as psy,
                tc.tile_pool(name="attn", bufs=3) as apool,
                tc.tile_pool(name="attn2", bufs=2) as apool2,
            ):
                for tq in range(N_TQ):
                    q0 = tq * TQC
                    for h in range(HPC):
                        qm, qp = h // 2, (h % 2) * 64
                        km = 2 + h // 2
                        o_ps = pso.tile([HD + 1, TQC], f32, tag="o")
                        for kt in range(N_KT):
                            s_ps = pss.tile([128, TQC], f32, tag="s")
                            nc.tensor.matmul(
                                s_ps[:],
                                qkvT[qp:qp + HD, km, kt * 128:(kt + 1) * 128],
                                qkvT[qp:qp + HD, qm, q0:q0 + TQC],
                                start=True, stop=True,
                            )
                            p_sb = apool.tile([128, TQC], bf16, tag="p")
                            nc.scalar.activation(p_sb[:], s_ps[:], Exp,
                                                 scale=1.0 / HD)
                            nc.tensor.matmul(
                                o_ps[:],
                                vaug[:, h, kt, :],
                                p_sb[:],
                                start=(kt == 0), stop=(kt == N_KT - 1),
                            )
                        o_sb = apool.tile([HD + 1, TQC], bf16, tag="osb")
                        nc.vector.tensor_copy(o_sb[:], o_ps[:])
                        om, op = h // 2, (h % 2) * 64
                        for sub in range(TQC // 128):
                            ot = pst.tile([128, HD + 1], bf16, tag="tr")
                            nc.tensor.transpose(
                                ot[:], o_sb[:, sub * 128:(sub + 1) * 128],
                                ident[0:HD + 1, 0:HD + 1])
                            rinv = apool2.tile([128, 1], f32, tag="rinv")
                            nc.vector.reciprocal(rinv[:], ot[:, HD:HD + 1])
                            onrm = apool2.tile([128, HD], bf16, tag="onrm")
                            nc.vector.tensor_scalar_mul(onrm[:], ot[:, 0:HD],
                                                        rinv[:])
                            ot2 = pst.tile([HD, 128], bf16, tag="tr")
                            nc.tensor.transpose(ot2[:], onrm[:], ident[:])
                            nc.vector.tensor_copy(
                                oallT[op:op + HD, om,
                                      q0 + sub * 128:q0 + (sub + 1) * 128],
                                ot2[:])
                    # proj for this chunk
                    for tt in range(TQC // 128):
                        y_sb = apool.tile([128, D], f32, tag="ysb")
                        for nn in range(2):
                            y_ps = psy.tile([128, 512], f32, tag="y")
                            for kc in range(2):
                                nc.tensor.matmul(
                                    y_ps[:],
                                    oallT[:, kc,
                                          q0 + tt * 128:q0 + (tt + 1) * 128],
                                    wproj[:, kc, nn * 512:(nn + 1) * 512],
                                    start=(kc == 0), stop=(kc == 1),
                                )
                            nc.vector.tensor_copy(
                                y_sb[:, nn * 512:(nn + 1) * 512], y_ps[:])
                        nc.sync.dma_start(
                            y_bounce[q0 + tt * 128:q0 + (tt + 1) * 128, :],
                            y_sb[:])
                    rs_out = drampool.tile([TQC // 4, D], f32, tag=f"rs{tq}")
                    nc.gpsimd.collective_compute(
                        "ReduceScatter", mybir.AluOpType.add,
                        replica_groups=GROUPS,
                        ins=[y_bounce[q0:q0 + TQC, :]],
                        outs=[rs_out[:]],
                    )
                    nc.sync.dma_start(out_ext.ap()[tq], rs_out[:])

    nc.compile()
    return nc


def _install_profile_hook():
    """Provide antenv.axon_hooks (absent in this image) so bass_utils'
    axon trace path can reach the NTFF profiler in libaxon_pjrt.so."""
    try:
        import antenv
        if "antenv.axon_hooks" not in sys.modules:
            mod = types.ModuleType("antenv.axon_hooks")
            mod._hook = None
            mod.set_axon_ntff_profile_hook = lambda h: setattr(mod, "_hook", h)
            mod.get_axon_ntff_profile_hook = lambda: mod._hook
            sys.modules["antenv.axon_hooks"] = mod
            antenv.axon_hooks = mod
        from trn_agent_boot.trn_boot import _ntff_profile_via_ctypes
        hook = _ntff_profile_via_ctypes("/opt/axon/libaxon_pjrt.so")
        sys.modules["antenv.axon_hooks"].set_axon_ntff_profile_hook(hook)
        return True
    except Exception:
        return False


def kernel(x, W_qkv, W_proj):
    global LAST_EXEC_NS
    x = np.asarray(x, dtype=np.float32)
    W_qkv = np.asarray(W_qkv, dtype=np.float32)
    W_proj = np.asarray(W_proj, dtype=np.float32)

    if "nc" not in _CACHE:
        _CACHE["nc"] = _build()
    nc = _CACHE["nc"]

    npbf16 = mybir.dt.np(bf16)
    xT = [np.ascontiguousarray(x[b].T).astype(npbf16) for b in range(B)]
    in_maps = []
    for c in range(N_CORES):
        b, g = c // 4, c % 4
        wq = W_qkv[:, g * DSH:(g + 1) * DSH]
        wk = W_qkv[:, D + g * DSH:D + (g + 1) * DSH]
        wv = W_qkv[:, 2 * D + g * DSH:2 * D + (g + 1) * DSH]
        in_maps.append({
            "xT": xT[b],
            "w_qkv": np.concatenate([wq, wk, wv], axis=1).astype(npbf16),
            "w_proj": np.ascontiguousarray(
                W_proj[g * DSH:(g + 1) * DSH, :]).astype(npbf16),
        })

    profile = bool(os.environ.get("BASS_KERNEL_PROFILE"))
    trace_dir = os.environ.get("BASS_KERNEL_TRACE_DIR") or None
    if profile:
        profile = _install_profile_hook()
    res = run_bass_kernel_spmd(
        nc, in_maps, core_ids=list(range(N_CORES)),
        trace=profile, tmpdir=trace_dir)
    LAST_EXEC_NS = res.exec_time_ns

    y = np.empty((B, T, D), dtype=np.float32)
    for c in range(N_CORES):
        b, r = c // 4, c % 4
        oc = res.results[c]["out"]
        for tq in range(N_TQ):
            y[b, tq * TQC + r * 128:tq * TQC + (r + 1) * 128, :] = oc[tq]
    return y


# revision 6
# speedup vs baseline: 1.2989x; 1.2989x over previous
"""Multi-head attention forward on 8 TRN2 NeuronCores.

Sharding: tensor-parallel over heads (4 groups of 4 heads) x data-parallel
over batch (2). Core c: batch c//4, heads [4*(c%4), 4*(c%4)+4).
Each 4-core batch group ReduceScatters the projection partials (bf16, 8
chunks, overlapped with compute) so every core ends with disjoint
[8, 64, 1024] slices of the final output; the host reassembles.

Compute layout is feature-major (transposed) throughout:
  qkvT = W_shard^T @ x^T          [768, T]   (PE, bf16 in / f32 psum)
  S^T  = kT^T qT per k-tile pair  [128, 1024] psum (two 512-col halves)
  P^T  = exp(S^T / 64)            (ScalarE, 1024 wide; no max-subtraction
                                   needed: scores have sigma ~0.125)
  O_aug^T = V_aug^T @ P^T accum   [65, 512]  (V_aug has a ones column so
                                   row 64 accumulates the softmax denom)
  transpose 128-col blocks of O_aug -> q on partitions -> reciprocal *
  scale on VectorE -> transpose back -> O_all^T
  y = O_all^T^T @ W_proj          [128, 512] psum tiles

The S->exp->O chain is software-pipelined: the next pair's S matmuls are
emitted before the previous pair's O matmuls so the in-order PE queue
never waits head-of-line on ScalarE's exp.
"""
import os
import sys
import types

import numpy as np

if "/opt/trn_rl_repo" not in sys.path:
    sys.path.insert(0, "/opt/trn_rl_repo")

import concourse.bass as bass
import concourse.bacc as bacc
import concourse.tile as tile
import concourse.mybir as mybir
from concourse import masks
from concourse.bass_utils import run_bass_kernel_spmd

B, T, D = 2, 2048, 1024
H, HD = 16, 64
N_CORES = 8
GROUPS = [[0, 1, 2, 3], [4, 5, 6, 7]]
HPC = 4                 # heads per core
DSH = HPC * HD          # 256 per-core head features
QKV_COLS = 3 * DSH      # 768
TQC = 512               # q-chunk
N_TQ = T // TQC         # 4
N_KT = T // 128         # 16 k-tiles
N_RS = 8                # reduce-scatter chunks (256 rows each)

f32 = mybir.dt.float32
bf16 = mybir.dt.bfloat16

LAST_EXEC_NS = None
_CACHE = {}


def _build():
    nc = bacc.Bacc("TRN2", target_bir_lowering=False, debug=False,
                   num_devices=N_CORES)
    xT_ext = nc.dram_tensor("xT", [D, T], bf16, kind="ExternalInput")
    wqkv_ext = nc.dram_tensor("w_qkv", [D, QKV_COLS], bf16,
                              kind="ExternalInput")
    wproj_ext = nc.dram_tensor("w_proj", [DSH, D], bf16, kind="ExternalInput")
    out_ext = nc.dram_tensor("out", [N_RS, T // 4 // N_RS, D], bf16,
                             kind="ExternalOutput")
    Exp = mybir.ActivationFunctionType.Exp

    with tile.TileContext(nc) as tc:
        with (
            tc.tile_pool(name="persist", bufs=1) as persist,
            tc.tile_pool(name="dram", bufs=1, space="DRAM") as drampool,
        ):
            ident = persist.tile([128, 128], bf16)
            masks.make_identity(nc, ident[:])

            qkvT = persist.tile([128, 6, T], bf16)   # rows m*128+p of [768,T]
            wproj = persist.tile([128, 2, D], bf16)  # rows of [256, 1024]
            oallT = persist.tile([128, 2, T], bf16)  # rows of O_all^T [256, T]
            vaug = persist.tile([128, HPC, N_KT, HD + 1], bf16)

            nc.sync.dma_start(
                wproj[:], wproj_ext.ap().rearrange("(c p) d -> p c d", p=128))

            y_bounce = drampool.tile([T, D], bf16, tag="ybounce")

            # ---- QKV (qkvT = W_qkv_shard^T @ x^T) + V_aug transposes ----
            with (
                tc.tile_pool(name="qkv_in", bufs=1) as qin,
                tc.tile_pool(name="ps_qkv", bufs=2, space="PSUM") as psq,
            ):
                xT = qin.tile([128, 8, T], bf16)
                wqkv = qin.tile([128, 8, QKV_COLS], bf16)
                nc.sync.dma_start(
                    wqkv[:], wqkv_ext.ap().rearrange("(k p) m -> p k m", p=128))
                xT_src = xT_ext.ap().rearrange("(k p) t -> p k t", p=128)
                for tch in range(N_TQ):
                    t0 = tch * TQC
                    nc.sync.dma_start(xT[:, :, t0:t0 + TQC],
                                      xT_src[:, :, t0:t0 + TQC])
                # v rows (m=4,5) first so V_aug transposes can start early
                for m in (4, 5, 0, 1, 2, 3):
                    for tch in range(N_TQ):
                        t0 = tch * TQC
                        ps = psq.tile([128, TQC], f32, tag="qkv")
                        for k in range(8):
                            nc.tensor.matmul(
                                ps[:],
                                wqkv[:, k, m * 128:(m + 1) * 128],
                                xT[:, k, t0:t0 + TQC],
                                start=(k == 0), stop=(k == 7),
                            )
                        nc.vector.tensor_copy(qkvT[:, m, t0:t0 + TQC], ps[:])
                nc.gpsimd.memset(vaug[:, :, :, HD:HD + 1], 1.0)
                for h in range(HPC):
                    vm, vp = 4 + h // 2, (h % 2) * 64
                    for kt in range(N_KT):
                        ptr = psq.tile([128, HD], bf16, tag="vtr")
                        nc.tensor.transpose(
                            ptr[:],
                            qkvT[vp:vp + HD, vm, kt * 128:(kt + 1) * 128],
                            ident[vp:vp + HD, vp:vp + HD])
                        nc.vector.tensor_copy(vaug[:, h, kt, 0:HD], ptr[:])

            # ---- attention + proj + chunked reduce-scatter ----
            with (
                tc.tile_pool(name="ps_s", bufs=2, space="PSUM") as pss,
                tc.tile_pool(name="ps_o", bufs=2, space="PSUM") as pso,
                tc.tile_pool(name="ps_m", bufs=2, space="PSUM") as psm,
                tc.tile_pool(name="attn", bufs=3) as apool,
                tc.tile_pool(name="attn2", bufs=2) as apool2,
            ):
                def epi_sub(tq, h, o_sb, sub):
                    """Normalize one 128-col block of head h's O into oallT."""
                    q0 = tq * TQC
                    om, op = h // 2, (h % 2) * 64
                    ot = psm.tile([128, HD + 1], bf16, tag="m")
                    nc.tensor.transpose(
                        ot[:], o_sb[:, sub * 128:(sub + 1) * 128],
                        ident[0:HD + 1, 0:HD + 1])
                    rinv = apool2.tile([128, 1], f32, tag="rinv")
                    nc.vector.reciprocal(rinv[:], ot[:, HD:HD + 1])
                    onrm = apool2.tile([128, HD], bf16, tag="onrm")
                    nc.vector.tensor_scalar_mul(onrm[:], ot[:, 0:HD], rinv[:])
                    ot2 = psm.tile([HD, 128], bf16, tag="m")
                    nc.tensor.transpose(ot2[:], onrm[:], ident[:])
                    nc.vector.tensor_copy(
                        oallT[op:op + HD, om,
                              q0 + sub * 128:q0 + (sub + 1) * 128],
                        ot2[:])

                def epilogue(tq, h, o_sb):
                    for sub in range(TQC // 128):
                        epi_sub(tq, h, o_sb, sub)

                pending = None   # deferred epilogue args
                for tq in range(N_TQ):
                    q0 = tq * TQC
                    for h in range(HPC):
                        qm, qp = h // 2, (h % 2) * 64
                        km = 2 + h // 2
                        o_ps = pso.tile([HD + 1, TQC], f32, tag="o")
                        prev_p = None
                        for j in range(N_KT // 2):   # k-tile pairs
                            s2 = pss.tile([128, 2 * TQC], f32, tag="s")
                            for half in range(2):
                                kt = 2 * j + half
                                nc.tensor.matmul(
                                    s2[:, half * TQC:(half + 1) * TQC],
                                    qkvT[qp:qp + HD, km,
                                         kt * 128:(kt + 1) * 128],
                                    qkvT[qp:qp + HD, qm, q0:q0 + TQC],
                                    start=True, stop=True,
                                )
                            p2 = apool.tile([128, 2 * TQC], bf16, tag="p")
                            nc.scalar.activation(p2[:], s2[:], Exp,
                                                 scale=1.0 / HD)
                            if prev_p is not None:
                                pj, pp = prev_p
                                for half in range(2):
                                    kt = 2 * pj + half
                                    nc.tensor.matmul(
                                        o_ps[:], vaug[:, h, kt, :],
                                        pp[:, half * TQC:(half + 1) * TQC],
                                        start=(kt == 0), stop=False,
                                    )
                            prev_p = (j, p2)
                        pj, pp = prev_p
                        for half in range(2):
                            kt = 2 * pj + half
                            nc.tensor.matmul(
                                o_ps[:], vaug[:, h, kt, :],
                                pp[:, half * TQC:(half + 1) * TQC],
                                start=False, stop=(kt == N_KT - 1),
                            )
                        o_sb = apool.tile([HD + 1, TQC], bf16, tag="osb")
                        nc.vector.tensor_copy(o_sb[:], o_ps[:])
                        if pending is not None:
                            epilogue(*pending)
                            pending = None
                        if h < HPC - 1:
                            pending = (tq, h, o_sb)
                        else:
                            last_osb = o_sb
                    # proj for this chunk; head 3's per-sub epilogue is
                    # interleaved just ahead of the proj tile that reads it
                    for tt in range(TQC // 128):
                        epi_sub(tq, HPC - 1, last_osb, tt)
                        y_sb = apool.tile([128, D], bf16, tag="ysb")
                        for nn in range(2):
                            y_ps = psm.tile([128, 512], f32, tag="m")
                            for kc in range(2):
                                nc.tensor.matmul(
                                    y_ps[:],
                                    oallT[:, kc,
                                          q0 + tt * 128:q0 + (tt + 1) * 128],
                                    wproj[:, kc, nn * 512:(nn + 1) * 512],
                                    start=(kc == 0), stop=(kc == 1),
                                )
                            nc.vector.tensor_copy(
                                y_sb[:, nn * 512:(nn + 1) * 512], y_ps[:])
                        nc.sync.dma_start(
                            y_bounce[q0 + tt * 128:q0 + (tt + 1) * 128, :],
                            y_sb[:])
                        if tt % 2 == 1:
                            gc = tq * 2 + tt // 2   # global 256-row chunk
                            rs_out = drampool.tile([256 // 4, D], bf16,
                                                   tag=f"rs{gc}")
                            nc.gpsimd.collective_compute(
                                "ReduceScatter", mybir.AluOpType.add,
                                replica_groups=GROUPS,
                                ins=[y_bounce[gc * 256:(gc + 1) * 256, :]],
                                outs=[rs_out[:]],
                            )
                            nc.sync.dma_start(out_ext.ap()[gc], rs_out[:])

    nc.compile()
    return nc


def _install_profile_hook():
    """Provide antenv.axon_hooks (absent in this image) so bass_utils'
    axon trace path can reach the NTFF profiler in libaxon_pjrt.so."""
    try:
        import antenv
        if "antenv.axon_hooks" not in sys.modules:
            mod = types.ModuleType("antenv.axon_hooks")
            mod._hook = None
            mod.set_axon_ntff_profile_hook = lambda h: setattr(mod, "_hook", h)
            mod.get_axon_ntff_profile_hook = lambda: mod._hook
            sys.modules["antenv.axon_hooks"] = mod
            antenv.axon_hooks = mod
        from trn_agent_boot.trn_boot import _ntff_profile_via_ctypes
        hook = _ntff_profile_via_ctypes("/opt/axon/libaxon_pjrt.so")
        sys.modules["antenv.axon_hooks"].set_axon_ntff_profile_hook(hook)
        return True
    except Exception:
        return False


def kernel(x, W_qkv, W_proj):
    global LAST_EXEC_NS
    x = np.asarray(x, dtype=np.float32)
    W_qkv = np.asarray(W_qkv, dtype=np.float32)
    W_proj = np.asarray(W_proj, dtype=np.float32)

    if "nc" not in _CACHE:
        _CACHE["nc"] = _build()
    nc = _CACHE["nc"]

    npbf16 = mybir.dt.np(bf16)
    xT = [np.ascontiguousarray(x[b].T).astype(npbf16) for b in range(B)]
    in_maps = []
    for c in range(N_CORES):
        b, g = c // 4, c % 4
        wq = W_qkv[:, g * DSH:(g + 1) * DSH]
        wk = W_qkv[:, D + g * DSH:D + (g + 1) * DSH]
        wv = W_qkv[:, 2 * D + g * DSH:2 * D + (g + 1) * DSH]
        in_maps.append({
            "xT": xT[b],
            "w_qkv": np.concatenate([wq, wk, wv], axis=1).astype(npbf16),
            "w_proj": np.ascontiguousarray(
                W_proj[g * DSH:(g + 1) * DSH, :]).astype(npbf16),
        })

    profile = bool(os.environ.get("BASS_KERNEL_PROFILE"))
    trace_dir = os.environ.get("BASS_KERNEL_TRACE_DIR") or None
    if profile:
        profile = _install_profile_hook()
    res = run_bass_kernel_spmd(
        nc, in_maps, core_ids=list(range(N_CORES)),
        trace=profile, tmpdir=trace_dir)
    LAST_EXEC_NS = res.exec_time_ns

    rows = T // 4 // N_RS   # 64 rows per rank per chunk
    y = np.empty((B, T, D), dtype=np.float32)
    for c in range(N_CORES):
        b, r = c // 4, c % 4
        oc = res.results[c]["out"].astype(np.float32)
        for gc in range(N_RS):
            y[b, gc * 256 + r * rows:gc * 256 + (r + 1) * rows, :] = oc[gc]
    return y
